# revision 1
# baseline (speedup 1.0000x reference)
import sys as _sys
for _p in ("/opt/trn_rl_repo", "/root/.axon_site/_ro/trn_rl_repo"):
    if _p not in _sys.path:
        _sys.path.insert(0, _p)
"""DGCNN Bass/Tile kernel for TRN2 — one NeuronCore handles one point cloud.

Layout conventions (per core):
  h_l: [C, 4096] f32 SBUF, channels on partitions (C in {3, 64, 64, 128}; l4 out 256 = 2 tiles)
  Edge conv trick: max_k LRelu(s*(W @ [xj-xi, xi]) + t)
      = LRelu( max_k(P'[:, j]) + q[:, i] )  with  P' = (s.Wd) @ h,  q = (s.(Wi-Wd)) @ h + (s.b + be)
  kNN: d[i, j] = 2<h_i, h_j> - |h_j|^2 (+ const(i), dropped; top-k invariant)
      via PE matmul with augmented operands, f32r.
  top-20: per 128-point block, per-row: 16 groups of 256 -> top-8 vals+idx (DVE max/max_index),
      theta = 20th of group-candidates, select, extract indices by value-max rounds.
  gather: gpsimd.ap_gather on SBUF with per-core replicated wrapped index lists
      (bounced through DRAM to linearize [128,20] -> flat p-major list).
"""
import numpy as np
import concourse.bass as bass
import concourse.tile as tile
from concourse import bacc, mybir, library_config

F32 = mybir.dt.float32
F32R = mybir.dt.float32r
I16 = mybir.dt.int16
U16 = mybir.dt.uint16
AF = mybir.ActivationFunctionType
ALU = mybir.AluOpType
AX = mybir.AxisListType

N = 4096
NBLK = 32
K = 20
SLOPE = 0.2
NEG = -1e30
H_DIM = (64, 64, 128, 256)
IN_CHAN = 3
W_DIM = 1024
CSUM = sum(H_DIM)  # 512


def build_kernel():
    nc = bacc.Bacc("TRN2", target_bir_lowering=False, debug=False)

    # ---- DRAM I/O ----
    h0_in = nc.dram_tensor("h0", [IN_CHAN, N], F32, kind="ExternalInput").ap()
    idx0_in = nc.dram_tensor("idx0", [NBLK, 128 * K], I16, kind="ExternalInput").ap()
    goff_in = nc.dram_tensor("goff", [128, 128], F32, kind="ExternalInput").ap()
    wp_in, wq_in, bq_in = [], [], []
    cins = [IN_CHAN, 64, 64, 128]
    for l in range(4):
        co = H_DIM[l]
        wp_in.append(nc.dram_tensor(f"wp{l}", [cins[l], co], F32, kind="ExternalInput").ap())
        wq_in.append(nc.dram_tensor(f"wq{l}", [cins[l], co], F32, kind="ExternalInput").ap())
        bq_in.append(nc.dram_tensor(f"bq{l}", [min(co, 128), (co + 127) // 128], F32,
                                    kind="ExternalInput").ap())
    wf_in = nc.dram_tensor("wf", [CSUM, W_DIM], F32, kind="ExternalInput").ap()
    bf_in = nc.dram_tensor("bf", [128, 8], F32, kind="ExternalInput").ap()
    y_out = nc.dram_tensor("y", [8, 128], F32, kind="ExternalOutput").ap()

    scr = nc.dram_tensor("scr", [NBLK, 128 * K], I16).ap()  # neighbor-list scratch

    with tile.TileContext(nc) as tc:
        with (
            tc.tile_pool(name="hpool", bufs=1) as hpool,      # persistent h tiles
            tc.tile_pool(name="wpool", bufs=1) as wpool,      # weights/constants
        ):
            nc.gpsimd.load_library(library_config.ap_gather)

            # persistent feature tiles
            h = [None] * 5
            h[0] = hpool.tile([IN_CHAN, N], F32, tag="h0", name="h0")
            nc.sync.dma_start(h[0][:], h0_in[:])
            h[1] = hpool.tile([64, N], F32, tag="h1", name="h1")
            h[2] = hpool.tile([64, N], F32, tag="h2", name="h2")
            h[3] = hpool.tile([128, N], F32, tag="h3", name="h3")
            h4a = hpool.tile([128, N], F32, tag="h4a", name="h4a")
            h4b = hpool.tile([128, N], F32, tag="h4b", name="h4b")

            gofft = wpool.tile([128, 128], F32, tag="goff", name="goff")
            nc.sync.dma_start(gofft[:], goff_in[:])

            # layer weights to SBUF
            wpt, wqt, bqt = [], [], []
            for l in range(4):
                ci, co = cins[l], H_DIM[l]
                t1 = wpool.tile([ci, co], F32, tag=f"wp{l}", name=f"wp{l}")
                nc.sync.dma_start(t1[:], wp_in[l][:])
                wpt.append(t1)
                t2 = wpool.tile([ci, co], F32, tag=f"wq{l}", name=f"wq{l}")
                nc.sync.dma_start(t2[:], wq_in[l][:])
                wqt.append(t2)
                t3 = wpool.tile([min(co, 128), (co + 127) // 128], F32, tag=f"bq{l}", name=f"bq{l}")
                nc.sync.dma_start(t3[:], bq_in[l][:])
                bqt.append(t3)

            # ================= per-layer machinery =================

            def knn_phase(l, hin, cin):
                """Compute neighbor lists for layer l (l>=1) into scr."""
                with (
                    tc.tile_pool(name=f"kA{l}", bufs=1) as pA,
                    tc.tile_pool(name=f"kAd{l}", bufs=2) as pAd,
                    tc.tile_pool(name=f"kAs{l}", bufs=3) as pS,
                ):
                    fused = cin < 128  # sq row rides in the aug tiles
                    caug = cin + 1 if fused else cin
                    # hsq = h*h (ACT), sq = ones^T @ hsq (PE)
                    hsq = pA.tile([cin, N], F32, tag="hsq", name="hsq")
                    nc.scalar.activation(hsq[:], hin[:], AF.Square)
                    onesc = pS.tile([cin, 1], F32, tag="onesc", name="onesc")
                    nc.vector.memset(onesc[:], 1.0)
                    sqrow = pA.tile([1, N], F32, tag="sqrow", name="sqrow")
                    with tc.tile_pool(name=f"kAps2{l}", bufs=2, space="PSUM") as pps2:
                        for c in range(8):
                            sq_ps = pps2.tile([1, 512], F32, tag="sqps", name="sqps")
                            nc.tensor.matmul(sq_ps[:], onesc[:],
                                             hsq[:, 512 * c:512 * (c + 1)],
                                             start=True, stop=True)
                            nc.scalar.activation(sqrow[:, 512 * c:512 * (c + 1)], sq_ps[:], AF.Copy)

                    # aug operands
                    rhs2h = pA.tile([caug, N], F32, tag="rhs2h", name="rhs2h")
                    nc.scalar.activation(rhs2h[0:cin, :], hin[:], AF.Copy, scale=2.0)
                    if fused:
                        lhs_aug = pA.tile([caug, N], F32, tag="lhsaug", name="lhsaug")
                        nc.scalar.activation(lhs_aug[0:cin, :], hin[:], AF.Copy)
                        nc.vector.memset(lhs_aug[cin:caug, :], -1.0)
                        # sq row into partition cin of rhs2h (partition shift => DMA)
                        nc.sync.dma_start(rhs2h[cin:caug, :], sqrow[:])
                    else:
                        lhs_aug = hin
                        negones = pS.tile([1, 128], F32, tag="negones", name="negones")
                        nc.vector.memset(negones[:], -1.0)

                    with tc.tile_pool(name=f"kAps{l}", bufs=2, space="PSUM") as pps:
                        for pb in range(NBLK):
                            blk = slice(128 * pb, 128 * (pb + 1))
                            d = pAd.tile([128, N], F32, tag="d", name="d")
                            for half in range(2):
                                Dh = pps.tile([128, N // 2], F32, tag="Dh", name="Dh")
                                for c in range(4):
                                    cs = slice(512 * c, 512 * (c + 1))
                                    gcs = slice(2048 * half + 512 * c,
                                                2048 * half + 512 * (c + 1))
                                    if fused:
                                        nc.tensor.matmul(Dh[:, cs], lhs_aug[:, blk],
                                                         rhs2h[:, gcs],
                                                         start=True, stop=True)
                                    else:
                                        nc.tensor.matmul(Dh[:, cs], lhs_aug[:, blk],
                                                         rhs2h[:, gcs],
                                                         start=True, stop=False)
                                        nc.tensor.matmul(Dh[:, cs], negones[:],
                                                         sqrow[:, gcs],
                                                         start=False, stop=True)
                                nc.scalar.activation(d[:, 2048 * half:2048 * (half + 1)],
                                                     Dh[:], AF.Copy)
                            topk_block(pb, d, pS)

            def topk_block(pb, d, pS):
                """Top-20 of each row of d [128, 4096]; writes scr[pb]."""
                gvals = pS.tile([128, 128], F32, tag="gvals", name="gvals")
                widx = pS.tile([128, 128], U16, tag="widx", name="widx")
                for g in range(16):
                    grp = d[:, 256 * g:256 * (g + 1)]
                    nc.vector.max(out=gvals[:, 8 * g:8 * (g + 1)], in_=grp)
                    nc.vector.max_index(out=widx[:, 8 * g:8 * (g + 1)],
                                        in_max=gvals[:, 8 * g:8 * (g + 1)], in_values=grp)
                gidxf = pS.tile([128, 128], F32, tag="gidxf", name="gidxf")
                nc.vector.tensor_copy(gidxf[:], widx[:])
                nc.vector.tensor_tensor(out=gidxf[:], in0=gidxf[:], in1=gofft[:], op=ALU.add)

                cp = pS.tile([128, 128], F32, tag="cp", name="cp")
                nc.vector.tensor_copy(cp[:], gvals[:])
                v24 = pS.tile([128, 24], F32, tag="v24", name="v24")
                for r in range(3):
                    nc.vector.max(out=v24[:, 8 * r:8 * (r + 1)], in_=cp[:])
                    if r < 2:
                        nc.vector.match_replace(out=cp[:], in_to_replace=v24[:, 8 * r:8 * (r + 1)],
                                                in_values=cp[:], imm_value=NEG)
                arr = pS.tile([128, 128], F32, tag="arr", name="arr")
                nc.vector.scalar_tensor_tensor(out=arr[:], in0=gvals[:], scalar=v24[:, 19:20],
                                               in1=gidxf[:], op0=ALU.is_ge, op1=ALU.mult)
                nc.vector.tensor_scalar(out=arr[:], in0=arr[:], scalar1=1.0, scalar2=None,
                                        op0=ALU.subtract)
                e24 = pS.tile([128, 24], F32, tag="e24", name="e24")
                for r in range(3):
                    nc.vector.max(out=e24[:, 8 * r:8 * (r + 1)], in_=arr[:])
                    if r < 2:
                        nc.vector.match_replace(out=arr[:], in_to_replace=e24[:, 8 * r:8 * (r + 1)],
                                                in_values=arr[:], imm_value=-1.0)
                i16t = pS.tile([128, K], I16, tag="i16t", name="i16t")
                nc.vector.tensor_copy(i16t[:], e24[:, 0:K])
                nc.sync.dma_start(scr[pb, :], i16t[:])

            def conv_phase(l, hin, cin, louts, idx_dram):
                """Projection + gather + k-max + LRelu for layer l.
                louts: list of (tile, C) output tiles (1 or 2 of up to 128 chans)."""
                co = H_DIM[l]
                ncts = len(louts)
                cgat = 128 if co >= 128 else co
                with (
                    tc.tile_pool(name=f"cB{l}", bufs=1) as pB,
                    tc.tile_pool(name=f"cBg{l}", bufs=3) as pG,
                    tc.tile_pool(name=f"cBs{l}", bufs=3) as pS,
                    tc.tile_pool(name=f"cBps{l}", bufs=4, space="PSUM") as pps,
                ):
                    # P' and q, full [co, N]
                    Pt, qt = [], []
                    for ct in range(ncts):
                        cw = louts[ct][1]
                        mt = slice(128 * ct, 128 * ct + cw)
                        Ptile = pB.tile([cw, N], F32, tag=f"P{ct}", name=f"P{ct}")
                        qtile = pB.tile([cw, N], F32, tag=f"q{ct}", name=f"q{ct}")
                        for c in range(8):
                            cs = slice(512 * c, 512 * (c + 1))
                            ps1 = pps.tile([cw, 512], F32, tag="pps1", name="pps1")
                            nc.tensor.matmul(ps1[:], wpt[l][:, mt],
                                             hin[:, cs], start=True, stop=True)
                            nc.scalar.activation(Ptile[:, cs], ps1[:], AF.Copy)
                            ps2 = pps.tile([cw, 512], F32, tag="pps2", name="pps2")
                            nc.tensor.matmul(ps2[:], wqt[l][:, mt],
                                             hin[:, cs], start=True, stop=True)
                            nc.scalar.activation(qtile[:, cs], ps2[:], AF.Identity,
                                                 bias=bqt[l][0:cw, ct:ct + 1])
                        Pt.append(Ptile)
                        qt.append(qtile)

                    for pb in range(NBLK):
                        blk = slice(128 * pb, 128 * (pb + 1))
                        # wrapped + replicated index list: idxrep[pp, 20*g+k] =
                        # scr_flat[20*(16g+pp)+k]  (list pos i = 16*(20g+k)+pp)
                        idxrep = pS.tile([cgat, 160], I16, tag="idxrep", name="idxrep")
                        s0 = idx_dram[pb, :]
                        wrapped = bass.AP(s0.tensor, s0.offset,
                                          [[20, 16], [320, 8], [1, 20]])
                        for g2 in range(cgat // 16):
                            dst = idxrep[16 * g2:16 * (g2 + 1), :]
                            dst3 = bass.AP(dst.tensor, dst.offset,
                                           [dst.ap[0], [20, 8], [1, 20]])
                            nc.sync.dma_start(dst3, wrapped)
                        for ct in range(ncts):
                            otile, cw = louts[ct]
                            gout = pG.tile([cgat, 128 * K], F32, tag="gout", name="gout")
                            nc.gpsimd.ap_gather(out_ap=gout[0:cw, :], in_ap=Pt[ct][:],
                                                idxs_ap=idxrep[0:cw, :], channels=cw,
                                                num_elems=N, d=1, num_idxs=128 * K)
                            M = pS.tile([cgat, 128], F32, tag="M", name="M")
                            nc.vector.tensor_reduce(
                                out=M[0:cw, :],
                                in_=gout[0:cw, :].rearrange("c (g k p) -> c g p k",
                                                            g=8, k=K, p=16),
                                axis=AX.X, op=ALU.max)
                            nc.vector.tensor_tensor(out=M[0:cw, :], in0=M[0:cw, :],
                                                    in1=qt[ct][:, blk], op=ALU.add)
                            nc.vector.scalar_tensor_tensor(out=otile[:, blk], in0=M[0:cw, :],
                                                           scalar=SLOPE, in1=M[0:cw, :],
                                                           op0=ALU.mult, op1=ALU.max)

            # ================= run the network =================
            conv_phase(0, h[0], IN_CHAN, [(h[1], 64)], idx0_in)
            knn_phase(1, h[1], 64)
            conv_phase(1, h[1], 64, [(h[2], 64)], scr)
            knn_phase(2, h[2], 64)
            conv_phase(2, h[2], 64, [(h[3], 128)], scr)
            knn_phase(3, h[3], 128)
            conv_phase(3, h[3], 128, [(h4a, 128), (h4b, 128)], scr)

            # ================= final 1x1 conv + global max =================
            with (
                tc.tile_pool(name="fw", bufs=1) as fw,
                tc.tile_pool(name="fs", bufs=3) as fs,
                tc.tile_pool(name="fps", bufs=4, space="PSUM") as fps,
            ):
                chunks = [(h[1], 0, 64), (h[2], 64, 64), (h[3], 128, 128),
                          (h4a, 256, 128), (h4b, 384, 128)]
                wf_t = []
                for (_, k0, kc) in chunks:
                    t = fw.tile([kc, W_DIM], F32, tag=f"wf{k0}", name=f"wf{k0}")
                    nc.sync.dma_start(t[:], wf_in[k0:k0 + kc, :])
                    wf_t.append(t)
                bft = fw.tile([128, 8], F32, tag="bft", name="bft")
                nc.sync.dma_start(bft[:], bf_in[:])

                for m in range(8):
                    mt = slice(128 * m, 128 * (m + 1))
                    ym = fs.tile([128, 8], F32, tag="ym", name="ym")
                    for c in range(8):
                        cs = slice(512 * c, 512 * (c + 1))
                        pf = fps.tile([128, 512], F32, tag="pf", name="pf")
                        for ci_, (ht, k0, kc) in enumerate(chunks):
                            nc.tensor.matmul(pf[:], wf_t[ci_][:, mt],
                                             ht[:, cs],
                                             start=(ci_ == 0), stop=(ci_ == len(chunks) - 1))
                        yt = fs.tile([128, 512], F32, tag="yt", name="yt")
                        nc.scalar.activation(yt[:], pf[:], AF.Identity,
                                             bias=bft[:, m:m + 1])
                        nc.vector.tensor_reduce(out=ym[:, c:c + 1], in_=yt[:],
                                                axis=AX.X, op=ALU.max)
                    yfin = fs.tile([128, 1], F32, tag="yfin", name="yfin")
                    nc.vector.tensor_reduce(out=yfin[:], in_=ym[:], axis=AX.X, op=ALU.max)
                    nc.vector.scalar_tensor_tensor(out=yfin[:], in0=yfin[:], scalar=SLOPE,
                                                   in1=yfin[:], op0=ALU.mult, op1=ALU.max)
                    nc.sync.dma_start(y_out[m, :], yfin[:])

    nc.compile()
    return nc


# ================= host-side input preparation =================

def prep_core_inputs(x_b, idx_b, params):
    """x_b [4096, 3] f32, idx_b [4096, 20] int, params = dict of W/b/g/be/Wf/bf."""
    BN_EPS = 1e-5
    inv = 1.0 / np.sqrt(1.0 + BN_EPS)
    inp = {}
    inp["h0"] = np.ascontiguousarray(x_b.T.astype(np.float32))
    inp["idx0"] = np.ascontiguousarray(idx_b.astype(np.int16).reshape(NBLK, 128 * K))
    inp["goff"] = np.broadcast_to(
        (np.arange(128) // 8 * 256 + 1).astype(np.float32)[None, :], (128, 128)).copy()
    cins = [IN_CHAN, 64, 64, 128]
    for l in range(4):
        W = params[f"W{l}"]; b = params[f"b{l}"]; g = params[f"g{l}"]; be = params[f"be{l}"]
        ci, co = cins[l], H_DIM[l]
        scale = (g * inv).astype(np.float32)
        Wd = W[:, :ci]
        Wi = W[:, ci:]
        Wp = scale[:, None] * Wd
        Wq = scale[:, None] * (Wi - Wd)
        bq = scale * b + be
        inp[f"wp{l}"] = np.ascontiguousarray(Wp.T.astype(np.float32))
        inp[f"wq{l}"] = np.ascontiguousarray(Wq.T.astype(np.float32))
        ncts = (co + 127) // 128
        bqm = np.zeros((min(co, 128), ncts), np.float32)
        for ct in range(ncts):
            cw = min(128, co - 128 * ct)
            bqm[:cw, ct] = bq[128 * ct:128 * ct + cw]
        inp[f"bq{l}"] = bqm
    inp["wf"] = np.ascontiguousarray(params["Wf"].T.astype(np.float32))
    inp["bf"] = np.ascontiguousarray(params["bf"].astype(np.float32).reshape(8, 128).T.copy())
    return inp


# ===================== public entry point =====================

_CACHED = {"nc": None}


def kernel(**inputs):
    """Full-input DGCNN forward. Shards batch over 8 NeuronCores."""
    x = np.asarray(inputs["x"])              # [8, 4096, 3] f32
    indices = np.asarray(inputs["indices"])  # [8, 4096, 20] int
    B = x.shape[0]
    params = {k: np.asarray(v) for k, v in inputs.items() if k not in ("x", "indices")}

    if _CACHED["nc"] is None:
        _CACHED["nc"] = build_kernel()
    nc = _CACHED["nc"]

    in_maps = [prep_core_inputs(x[b], indices[b], params) for b in range(B)]
    from concourse.bass_utils import run_bass_kernel_spmd
    res = run_bass_kernel_spmd(nc, in_maps, core_ids=list(range(B)))
    y = np.stack([res.results[b]["y"].reshape(-1) for b in range(B)])
    return y.astype(np.float32)

